# revision 13
# baseline (speedup 1.0000x reference)
r"""DbrxAttention on 8 TRN2 NeuronCores, tensor-parallel across heads.

Per-core shard (core c of 8): 6 query heads (q heads 6c..6c+5), kv head c
(replicated per its 6-head query group), plus the matching 768 input
columns of the out-projection. Each core computes a partial out-proj
(row-parallel Wout); the partials are summed on the host (the all-reduce
of the TP pattern).

Layouts (per core, all device tensors):
  hidT   [6144, 2048] fp16  hidden^T       (d on partitions)
  wqkvT  [6144, 1024] fp16  [q0..q5 | k | v] columns of Wqkv^T shard
  woutT  [768,  6144] fp16  Wout[:, shard]^T
  cos/sin tables [128, 2048] fp16, neox rope with sign-folded sin and the
  1/sqrt(128) score scale folded into the q tables.
  masks  [4, 128, 512] fp16  multiplicative causal masks for the four
         diagonal-straddle patterns of (128-wide kt tile, 512-wide qt chunk)

Pipeline: QKV GEMM (fp16, PSUM fp32) -> clip -> rope (DVE fp16 +
partition-shift DMA) -> per 512-wide q chunk jc and head h, a chain of
kt steps: scores^T = k^T.T @ q^T (trapezoid-narrowed on the diagonal),
exp on ACT into fp16 probs, causal mask multiply on diagonal blocks, row
sums via ones-matmul + attn^T accumulation via v-matmul. All chains of a
jc group are flattened into one software-pipelined stream, interleaved
with out-proj (t tiles 4(jc-1)..4jc-1 of the previous group) so the PE
never drains on exp latency. Normalization: reciprocal_approx_fast on
the PSUM row sums, gpsimd partition broadcast, DVE multiply straight
from PSUM into fp16 attnT. Partial [2048, 6144] fp32 out, summed across
the 8 cores on the host.
"""

import os

import ml_dtypes
import numpy as np

import concourse.mybir as mybir
import concourse.tile as tile
from concourse import bacc
from concourse.bass_utils import run_bass_kernel_spmd

F32 = mybir.dt.float32
F32R = mybir.dt.float32r
F16 = mybir.dt.float16
BF16 = mybir.dt.bfloat16

T = 2048
D = 6144
N_HEADS = 48
N_KV = 8
HD = 128
CLIP = 8.0
THETA = 500000.0
N_CORES = 8
HPC = N_HEADS // N_CORES      # q heads per core = 6
QKJ = HPC + 1                 # q+k j-tiles per core = 7
DCH = D // 128                # 48 contraction chunks
TCH = T // 512                # 4 t-chunks
TTILES = T // 128             # 16 t-tiles
OCH = D // 512                # 12 out-proj column chunks
ICH = HPC                     # 6 out-proj contraction chunks (768/128)

_compiled = None


def _build():
    nc = bacc.Bacc("TRN2", target_bir_lowering=False, debug=False,
                   num_devices=N_CORES)

    hidT_d = nc.dram_tensor("hidT", [D, T], F16, kind="ExternalInput").ap()
    wqkvT_d = nc.dram_tensor("wqkvT", [D, 1024], F16, kind="ExternalInput").ap()
    woutT_d = nc.dram_tensor("woutT", [HPC * HD, D], F16, kind="ExternalInput").ap()
    cosq_d = nc.dram_tensor("cosq", [HD, T], F16, kind="ExternalInput").ap()
    sinq_d = nc.dram_tensor("sinq", [HD, T], F16, kind="ExternalInput").ap()
    cosk_d = nc.dram_tensor("cosk", [HD, T], F16, kind="ExternalInput").ap()
    sink_d = nc.dram_tensor("sink", [HD, T], F16, kind="ExternalInput").ap()
    mask_d = nc.dram_tensor("maskm", [4, HD, 512], BF16, kind="ExternalInput").ap()
    ones_d = nc.dram_tensor("ones", [HD, 8], BF16, kind="ExternalInput").ap()
    outp_d = nc.dram_tensor("outp", [T, D], F32, kind="ExternalOutput").ap()

    mn, mx = mybir.AluOpType.min, mybir.AluOpType.max
    mult, add = mybir.AluOpType.mult, mybir.AluOpType.add
    EXP = mybir.ActivationFunctionType.Exp

    with tile.TileContext(nc) as tc:
        with (
            tc.tile_pool(name="sb", bufs=1) as pool,
            tc.tile_pool(name="ps", bufs=1, space="PSUM") as psum,
        ):
            # persistent tensors
            qkT = pool.tile([128, QKJ, T], F16)       # roped q (scaled) + k
            v_sb = pool.tile([128, TTILES, HD], BF16)   # clipped v, [t%128, t//128, hd]
            attnT = pool.tile([128, HPC, T], F16)      # normalized attn^T
            cosq = pool.tile([HD, T], F16)
            sinq = pool.tile([HD, T], F16)
            cosk = pool.tile([HD, T], F16)
            sink = pool.tile([HD, T], F16)
            masks = pool.tile([HD, 4, 512], BF16)
            ones = pool.tile([HD, 8], BF16)

            def load_tables_a():
                nc.sync.dma_start(cosq[:], cosq_d[:])
                nc.sync.dma_start(sinq[:], sinq_d[:])
                nc.sync.dma_start(cosk[:], cosk_d[:])
                nc.sync.dma_start(sink[:], sink_d[:])

            def load_tables_b():
                nc.sync.dma_start(masks[:], mask_d.rearrange("a p t -> p a t"))
                nc.sync.dma_start(ones[:], ones_d[:])

            def qkv_sweep(tcx, defer_ropes=False, after_d=None):
                tsl = slice(tcx * 512, (tcx + 1) * 512)
                qk_ps = [psum.tile([128, 512], F32, tag="bank", bufs=8,
                                   name=f"qk_ps{j}")
                         for j in range(QKJ)]
                v_ps = psum.tile([128, 512], F32, tag="bank", bufs=8)
                for d in range(DCH):
                    dsl = slice(d * 128, (d + 1) * 128)
                    hid = pool.tile([128, 512], F16, tag="hid", bufs=12)
                    wq = pool.tile([128, 1024], F16, tag="wq", bufs=12)
                    nc.sync.dma_start(hid[:], hidT_d[dsl, tsl])
                    nc.sync.dma_start(wq[:], wqkvT_d[dsl, :])
                    if after_d is not None and d in after_d:
                        after_d[d]()
                    st, sp = d == 0, d == DCH - 1
                    for j in range(QKJ):
                        nc.tensor.matmul(qk_ps[j][:], wq[:, j * 128:(j + 1) * 128],
                                         hid[:], start=st, stop=sp)
                    for s in range(4):
                        # packed quarter-bank outputs: start=True zeroes the
                        # whole 2KB zero-region, so only the first sub-matmul
                        # of the bank may set it
                        nc.tensor.matmul(v_ps[:, s * 128:(s + 1) * 128],
                                         hid[:, s * 128:(s + 1) * 128],
                                         wq[:, 896:1024],
                                         start=(st and s == 0),
                                         stop=(sp and s == 3),
                                         skip_group_check=True)
                # evacuate: all clips first (each clip releases a PSUM bank
                # for subsequent matmuls), ropes after (they only read the
                # SBUF raw tiles)
                raws = []
                for j in range(QKJ):
                    raw = pool.tile([128, 512], F16, tag="raw", bufs=8,
                                    name=f"raw{j}")
                    nc.vector.tensor_scalar(raw[:], qk_ps[j][:], CLIP, -CLIP, mn, mx)
                    raws.append(raw)
                nc.vector.tensor_scalar(
                    v_sb[:, tcx * 4:(tcx + 1) * 4, :],
                    v_ps[:].rearrange("p (a h) -> p a h", a=4),
                    CLIP, -CLIP, mn, mx)

                def do_ropes():
                    for j in [HPC] + list(range(HPC)):
                        raw = raws[j]
                        xr = pool.tile([128, 512], F16, tag="xr", bufs=4)
                        nc.sync.dma_start(xr[0:64, :], raw[64:128, :])
                        nc.sync.dma_start(xr[64:128, :], raw[0:64, :])
                        cosT = cosq if j < HPC else cosk
                        sinT = sinq if j < HPC else sink
                        dst = qkT[:, j, tsl]
                        nc.vector.tensor_tensor(dst, raw[:], cosT[:, tsl], mult)
                        nc.vector.tensor_tensor(xr[:], xr[:], sinT[:, tsl], mult)
                        nc.vector.tensor_tensor(dst, dst, xr[:], add)

                if defer_ropes:
                    return do_ropes
                do_ropes()
                return None

            # ---- out-proj unit emitters ----
            def op_unit(oc, t, wo, use_dve):
                osl = slice(oc * 512, (oc + 1) * 512)
                out_ps = psum.tile([128, 512], F32, tag="bank", bufs=8)
                for i in range(ICH):
                    nc.tensor.matmul(out_ps[:],
                                     attnT[:, i, t * 128:(t + 1) * 128],
                                     wo[:, i, :], start=(i == 0),
                                     stop=(i == ICH - 1))
                osb = pool.tile([128, 512], F32, tag="osb", bufs=6)
                if use_dve:
                    nc.vector.tensor_scalar_add(osb[:], out_ps[:], 0.0)
                else:
                    nc.scalar.copy(osb[:], out_ps[:])
                nc.sync.dma_start(outp_d[t * 128:(t + 1) * 128, osl], osb[:])

            def op_group_units(g):
                """(prefetch, emitters) for out-proj t-tiles 4g..4g+3; the
                wo weight DMA is prefetched one oc ahead, and the PSUM
                evacuation copy alternates between ACT and DVE."""
                units = []
                wo_tiles = {}

                def load_wo(oc):
                    osl = slice(oc * 512, (oc + 1) * 512)
                    wo = pool.tile([128, ICH, 512], F16, tag="wo", bufs=3)
                    nc.sync.dma_start(wo[:], woutT_d[:, osl].rearrange(
                        "(i p) o -> p i o", p=128))
                    wo_tiles[oc] = wo

                def make(k, oc, t, prefetch_oc):
                    def emit():
                        if oc not in wo_tiles:
                            load_wo(oc)
                        if oc + 1 < OCH and oc + 1 not in wo_tiles:
                            load_wo(oc + 1)
                        if prefetch_oc is not None and prefetch_oc not in wo_tiles:
                            load_wo(prefetch_oc)
                        op_unit(oc, t, wo_tiles[oc], use_dve=(k % 2 == 1))
                    return emit

                k = 0
                for oc in range(OCH):
                    for ti, t in enumerate(range(4 * g, 4 * g + 4)):
                        # prefetch two oc ahead as soon as oc starts
                        pf = oc + 2 if (ti == 0 and oc + 2 < OCH) else None
                        units.append(make(k, oc, t, pf))
                        k += 1

                def prefetch():
                    if 0 not in wo_tiles:
                        load_wo(0)
                    if 1 not in wo_tiles and OCH > 1:
                        load_wo(1)
                return prefetch, units

            # ---- flattened, software-pipelined attention chains for one
            # jc group, with out-proj units interleaved ----
            def emit_group(jc, op_group):
                prefetch_wo, op_units = op_group
                n_kt = 4 * jc + 4
                qsl = slice(jc * 512, (jc + 1) * 512)
                steps = [(h, kt) for h in range(HPC) for kt in range(n_kt)]
                nsteps = len(steps)
                nops = len(op_units)

                chain_ps = {}      # h -> (attn_ps, sums_ps)
                pending = []       # (h, kt, pb, width_off)
                deferred = []      # [countdown, callable]

                def tick():
                    for item in deferred:
                        item[0] -= 1
                    while deferred and deferred[0][0] <= 0:
                        deferred.pop(0)[1]()

                def flush_deferred():
                    while deferred:
                        deferred.pop(0)[1]()

                def emit_front(h, kt):
                    r = kt - 4 * jc
                    o = 0 if r < 0 else 128 * r
                    sc = psum.tile([128, 512], F32, tag="bank", bufs=8)
                    nc.tensor.matmul(sc[:, o:],
                                     qkT[:, HPC, kt * 128:(kt + 1) * 128],
                                     qkT[:, h, jc * 512 + o:(jc + 1) * 512],
                                     start=True, stop=True)
                    pb = pool.tile([128, 512], BF16, tag="pb", bufs=12)
                    nc.scalar.activation(pb[:, o:], sc[:, o:], EXP)
                    if r >= 0:
                        nc.vector.tensor_tensor(pb[:, o:], pb[:, o:],
                                                masks[:, r, o:], mult)
                    pending.append((h, kt, pb, o))

                def emit_back_batch():
                    batch = pending[:4]
                    del pending[:4]
                    h = batch[0][0]
                    if batch[0][1] == 0:
                        attn_ps = psum.tile([128, 512], F32, tag="bank", bufs=8,
                                            name="attn_ps")
                        sums_ps = psum.tile([128, 512], F32, tag="bank", bufs=8,
                                            name="sums_ps")
                        chain_ps[h] = (attn_ps, sums_ps)
                    attn_ps, sums_ps = chain_ps[h]
                    if jc >= 1:
                        # 4 row-sum matmuls packed into distinct 32-wide
                        # column groups of the PE array -- they run
                        # concurrently, each accumulating its kt mod 4
                        # subset into partition 32*g
                        for _, kt, pb, o in batch:
                            g = kt % 4
                            nc.tensor.matmul(sums_ps[32 * g:32 * g + 1, o:],
                                             ones[:, 0:1], pb[:, o:],
                                             start=(kt < 4),
                                             stop=(kt >= n_kt - 4),
                                             skip_group_check=True,
                                             tile_position=(0, 32 * g))
                    else:
                        for _, kt, pb, o in batch:
                            nc.tensor.matmul(sums_ps[0:1, o:],
                                             ones[:, 0:1], pb[:, o:],
                                             start=(kt == 0),
                                             stop=(kt == n_kt - 1),
                                             skip_group_check=True)
                    for _, kt, pb, o in batch:
                        nc.tensor.matmul(attn_ps[:, o:], v_sb[:, kt, :], pb[:, o:],
                                         start=(kt == 0), stop=(kt == n_kt - 1),
                                         skip_group_check=True)
                    if batch[-1][1] == n_kt - 1:
                        if jc >= 1:
                            # chain end: fold the 4 group rows (DVE may read
                            # only one PSUM operand per op), then 1/x
                            t0 = pool.tile([1, 512], F32, tag="fold", bufs=6)
                            nc.vector.tensor_scalar_add(t0[:], sums_ps[0:1, :],
                                                        0.0)
                            t1 = pool.tile([1, 512], F32, tag="fold", bufs=6)
                            nc.vector.tensor_tensor(t1[:], sums_ps[32:33, :],
                                                    t0[:], add)
                            t2 = pool.tile([1, 512], F32, tag="fold", bufs=6)
                            nc.vector.tensor_tensor(t2[:], sums_ps[64:65, :],
                                                    t1[:], add)
                            dsum = pool.tile([1, 512], F32, tag="fold", bufs=6)
                            nc.vector.tensor_tensor(dsum[:], sums_ps[96:97, :],
                                                    t2[:], add)
                        else:
                            dsum = sums_ps[0:1, :]
                        rec = pool.tile([1, 512], F32, tag="rec", bufs=2)
                        nc.vector.reciprocal_approx_fast(rec[:], dsum[:])
                        recb = pool.tile([128, 512], F32, tag="recb", bufs=3)
                        nc.gpsimd.partition_broadcast(recb[:], rec[:])

                        def norm(h=h, attn_ps=attn_ps, recb=recb):
                            nc.vector.tensor_tensor(attnT[:, h, qsl],
                                                    attn_ps[:], recb[:], mult)
                        deferred.append([3, norm])

                prefetch_wo()
                opi = 0
                for i, (h, kt) in enumerate(steps):
                    emit_front(h, kt)
                    tick()
                    # evenly distribute the op units among the steps
                    want = (i + 1) * nops // nsteps
                    while opi < want:
                        op_units[opi]()
                        opi += 1
                        tick()
                    while len(pending) >= 8:
                        emit_back_batch()
                        tick()
                while pending:
                    emit_back_batch()
                    tick()
                while opi < nops:
                    op_units[opi]()
                    opi += 1
                flush_deferred()

            # ---- emission order ----
            qkv_sweep(0, after_d={38: load_tables_a})
            qkv_sweep(1, after_d={44: load_tables_b})
            qkv_sweep(2)
            ropes3 = qkv_sweep(3, defer_ropes=True)
            emit_group(0, (lambda: None, []))          # jc=0 chains (need only sweep 0)
            ropes3()                   # sweep 3 ropes (needed by jc=3)
            emit_group(1, op_group_units(0))
            emit_group(2, op_group_units(1))
            emit_group(3, op_group_units(2))
            pf3, units3 = op_group_units(3)
            pf3()
            for u in units3:
                u()

    nc.compile()
    return nc


def kernel(hidden_states, position_ids, Wqkv, Wout):
    global _compiled
    hidden_states = np.asarray(hidden_states, dtype=np.float32)
    position_ids = np.asarray(position_ids).astype(np.int64)
    Wqkv = np.asarray(Wqkv, dtype=np.float32)
    Wout = np.asarray(Wout, dtype=np.float32)

    if _compiled is None:
        _compiled = _build()
    nc = _compiled

    # host prep: rope tables (from actual position_ids), masks, shards
    scale = HD ** -0.5
    half = HD // 2
    inv_freq = 1.0 / (THETA ** (np.arange(half, dtype=np.float64) / half))
    freqs = position_ids.astype(np.float64)[None, :] * inv_freq[:, None]  # [64, T]
    cos = np.cos(freqs)
    sin = np.sin(freqs)
    cosf = np.concatenate([cos, cos], 0)
    sinf = np.concatenate([-sin, sin], 0)
    cosq = (cosf * scale).astype(np.float16)
    sinq = (sinf * scale).astype(np.float16)
    cosk = cosf.astype(np.float16)
    sink = sinf.astype(np.float16)

    p = np.arange(128)[:, None]
    f = np.arange(512)[None, :]
    masks = np.stack([(f >= 128 * r + p) for r in range(4)]).astype(ml_dtypes.bfloat16)

    hidT = np.ascontiguousarray(hidden_states.T).astype(np.float16)
    ones = np.ones((HD, 8), ml_dtypes.bfloat16)

    q_size = N_HEADS * HD
    in_maps = []
    for c in range(N_CORES):
        qrows = Wqkv[c * HPC * HD:(c + 1) * HPC * HD]
        krows = Wqkv[q_size + c * HD:q_size + (c + 1) * HD]
        vrows = Wqkv[q_size + N_KV * HD + c * HD:q_size + N_KV * HD + (c + 1) * HD]
        wqkvT = np.ascontiguousarray(
            np.concatenate([qrows, krows, vrows], 0).T).astype(np.float16)
        woutT = np.ascontiguousarray(
            Wout[:, c * HPC * HD:(c + 1) * HPC * HD].T).astype(np.float16)
        in_maps.append({
            "hidT": hidT, "wqkvT": wqkvT, "woutT": woutT,
            "cosq": cosq, "sinq": sinq, "cosk": cosk, "sink": sink,
            "maskm": masks, "ones": ones,
        })

    trace = os.environ.get("DBRX_TRACE", "0") == "1"
    res = run_bass_kernel_spmd(nc, in_maps, core_ids=list(range(N_CORES)),
                               trace=trace)
    kernel.last_result = res

    out = res.results[0]["outp"].astype(np.float32)
    for c in range(1, N_CORES):
        out += res.results[c]["outp"]
    return out


# revision 14
# speedup vs baseline: 1.0055x; 1.0055x over previous
r"""DbrxAttention on 8 TRN2 NeuronCores, tensor-parallel across heads.

Per-core shard (core c of 8): 6 query heads (q heads 6c..6c+5), kv head c
(replicated per its 6-head query group), plus the matching 768 input
columns of the out-projection. Each core computes a partial out-proj
(row-parallel Wout); the partials are summed on the host (the all-reduce
of the TP pattern).

Layouts (per core, all device tensors):
  hidT   [6144, 2048] fp16  hidden^T       (d on partitions)
  wqkvT  [6144, 1024] fp16  [q0..q5 | k | v] columns of Wqkv^T shard
  woutT  [768,  6144] fp16  Wout[:, shard]^T
  cos/sin tables [128, 2048] fp16, neox rope with sign-folded sin and the
  1/sqrt(128) score scale folded into the q tables.
  masks  [4, 128, 512] fp16  multiplicative causal masks for the four
         diagonal-straddle patterns of (128-wide kt tile, 512-wide qt chunk)

Pipeline: QKV GEMM (fp16, PSUM fp32) -> clip -> rope (DVE fp16 +
partition-shift DMA) -> per 512-wide q chunk jc and head h, a chain of
kt steps: scores^T = k^T.T @ q^T (trapezoid-narrowed on the diagonal),
exp on ACT into fp16 probs, causal mask multiply on diagonal blocks, row
sums via ones-matmul + attn^T accumulation via v-matmul. All chains of a
jc group are flattened into one software-pipelined stream, interleaved
with out-proj (t tiles 4(jc-1)..4jc-1 of the previous group) so the PE
never drains on exp latency. Normalization: reciprocal_approx_fast on
the PSUM row sums, gpsimd partition broadcast, DVE multiply straight
from PSUM into fp16 attnT. Partial [2048, 6144] fp32 out, summed across
the 8 cores on the host.
"""

import os

import ml_dtypes
import numpy as np

import concourse.mybir as mybir
import concourse.tile as tile
from concourse import bacc
from concourse.bass_utils import run_bass_kernel_spmd

F32 = mybir.dt.float32
F32R = mybir.dt.float32r
F16 = mybir.dt.float16
BF16 = mybir.dt.bfloat16

T = 2048
D = 6144
N_HEADS = 48
N_KV = 8
HD = 128
CLIP = 8.0
THETA = 500000.0
N_CORES = 8
HPC = N_HEADS // N_CORES      # q heads per core = 6
QKJ = HPC + 1                 # q+k j-tiles per core = 7
DCH = D // 128                # 48 contraction chunks
TCH = T // 512                # 4 t-chunks
TTILES = T // 128             # 16 t-tiles
OCH = D // 512                # 12 out-proj column chunks
ICH = HPC                     # 6 out-proj contraction chunks (768/128)

_compiled = None


def _build():
    nc = bacc.Bacc("TRN2", target_bir_lowering=False, debug=False,
                   num_devices=N_CORES)

    hidT_d = nc.dram_tensor("hidT", [D, T], F16, kind="ExternalInput").ap()
    wqkvT_d = nc.dram_tensor("wqkvT", [D, 1024], F16, kind="ExternalInput").ap()
    woutT_d = nc.dram_tensor("woutT", [HPC * HD, D], F16, kind="ExternalInput").ap()
    cosq_d = nc.dram_tensor("cosq", [HD, T], F16, kind="ExternalInput").ap()
    sinq_d = nc.dram_tensor("sinq", [HD, T], F16, kind="ExternalInput").ap()
    cosk_d = nc.dram_tensor("cosk", [HD, T], F16, kind="ExternalInput").ap()
    sink_d = nc.dram_tensor("sink", [HD, T], F16, kind="ExternalInput").ap()
    mask_d = nc.dram_tensor("maskm", [4, HD, 512], BF16, kind="ExternalInput").ap()
    ones_d = nc.dram_tensor("ones", [HD, 8], BF16, kind="ExternalInput").ap()
    outp_d = nc.dram_tensor("outp", [T, D], F32, kind="ExternalOutput").ap()

    mn, mx = mybir.AluOpType.min, mybir.AluOpType.max
    mult, add = mybir.AluOpType.mult, mybir.AluOpType.add
    EXP = mybir.ActivationFunctionType.Exp

    with tile.TileContext(nc) as tc:
        with (
            tc.tile_pool(name="sb", bufs=1) as pool,
            tc.tile_pool(name="ps", bufs=1, space="PSUM") as psum,
        ):
            # persistent tensors
            qkT = pool.tile([128, QKJ, T], F16)       # roped q (scaled) + k
            v_sb = pool.tile([128, TTILES, HD], BF16)   # clipped v, [t%128, t//128, hd]
            attnT = pool.tile([128, HPC, T], F16)      # normalized attn^T
            cosq = pool.tile([HD, T], F16)
            sinq = pool.tile([HD, T], F16)
            cosk = pool.tile([HD, T], F16)
            sink = pool.tile([HD, T], F16)
            masks = pool.tile([HD, 4, 512], BF16)
            ones = pool.tile([HD, 8], BF16)

            def load_tables_a():
                nc.sync.dma_start(cosq[:], cosq_d[:])
                nc.sync.dma_start(sinq[:], sinq_d[:])
                nc.sync.dma_start(cosk[:], cosk_d[:])
                nc.sync.dma_start(sink[:], sink_d[:])

            def load_tables_b():
                nc.sync.dma_start(masks[:], mask_d.rearrange("a p t -> p a t"))
                nc.sync.dma_start(ones[:], ones_d[:])

            def qkv_sweep(tcx, defer_ropes=False, after_d=None):
                tsl = slice(tcx * 512, (tcx + 1) * 512)
                qk_ps = [psum.tile([128, 512], F32, tag="bank", bufs=8,
                                   name=f"qk_ps{j}")
                         for j in range(QKJ)]
                v_ps = psum.tile([128, 512], F32, tag="bank", bufs=8)
                for d in range(DCH):
                    dsl = slice(d * 128, (d + 1) * 128)
                    hid = pool.tile([128, 512], F16, tag="hid", bufs=12)
                    wq = pool.tile([128, 1024], F16, tag="wq", bufs=12)
                    nc.sync.dma_start(hid[:], hidT_d[dsl, tsl])
                    nc.sync.dma_start(wq[:], wqkvT_d[dsl, :])
                    if after_d is not None and d in after_d:
                        after_d[d]()
                    st, sp = d == 0, d == DCH - 1
                    for j in range(QKJ):
                        nc.tensor.matmul(qk_ps[j][:], wq[:, j * 128:(j + 1) * 128],
                                         hid[:], start=st, stop=sp)
                    for s in range(4):
                        # packed quarter-bank outputs: start=True zeroes the
                        # whole 2KB zero-region, so only the first sub-matmul
                        # of the bank may set it
                        nc.tensor.matmul(v_ps[:, s * 128:(s + 1) * 128],
                                         hid[:, s * 128:(s + 1) * 128],
                                         wq[:, 896:1024],
                                         start=(st and s == 0),
                                         stop=(sp and s == 3),
                                         skip_group_check=True)
                # evacuate: all clips first (each clip releases a PSUM bank
                # for subsequent matmuls), ropes after (they only read the
                # SBUF raw tiles)
                raws = []
                for j in range(QKJ):
                    raw = pool.tile([128, 512], F16, tag="raw", bufs=8,
                                    name=f"raw{j}")
                    nc.vector.tensor_scalar(raw[:], qk_ps[j][:], CLIP, -CLIP, mn, mx)
                    raws.append(raw)
                nc.vector.tensor_scalar(
                    v_sb[:, tcx * 4:(tcx + 1) * 4, :],
                    v_ps[:].rearrange("p (a h) -> p a h", a=4),
                    CLIP, -CLIP, mn, mx)

                def do_ropes():
                    for j in [HPC] + list(range(HPC)):
                        raw = raws[j]
                        xr = pool.tile([128, 512], F16, tag="xr", bufs=4)
                        nc.sync.dma_start(xr[0:64, :], raw[64:128, :])
                        nc.sync.dma_start(xr[64:128, :], raw[0:64, :])
                        cosT = cosq if j < HPC else cosk
                        sinT = sinq if j < HPC else sink
                        dst = qkT[:, j, tsl]
                        nc.vector.tensor_tensor(dst, raw[:], cosT[:, tsl], mult)
                        nc.vector.tensor_tensor(xr[:], xr[:], sinT[:, tsl], mult)
                        nc.vector.tensor_tensor(dst, dst, xr[:], add)

                if defer_ropes:
                    return do_ropes
                do_ropes()
                return None

            # ---- out-proj unit emitters ----
            def op_unit(oc, t, wo, use_dve):
                osl = slice(oc * 512, (oc + 1) * 512)
                out_ps = psum.tile([128, 512], F32, tag="bank", bufs=8)
                for i in range(ICH):
                    nc.tensor.matmul(out_ps[:],
                                     attnT[:, i, t * 128:(t + 1) * 128],
                                     wo[:, i, :], start=(i == 0),
                                     stop=(i == ICH - 1))
                osb = pool.tile([128, 512], F32, tag="osb", bufs=6)
                if use_dve:
                    nc.vector.tensor_scalar_add(osb[:], out_ps[:], 0.0)
                else:
                    nc.scalar.copy(osb[:], out_ps[:])
                nc.scalar.dma_start(outp_d[t * 128:(t + 1) * 128, osl], osb[:])

            def op_group_units(g):
                """(prefetch, emitters) for out-proj t-tiles 4g..4g+3; the
                wo weight DMA is prefetched one oc ahead, and the PSUM
                evacuation copy alternates between ACT and DVE."""
                units = []
                wo_tiles = {}

                def load_wo(oc):
                    osl = slice(oc * 512, (oc + 1) * 512)
                    wo = pool.tile([128, ICH, 512], F16, tag="wo", bufs=3)
                    nc.sync.dma_start(wo[:], woutT_d[:, osl].rearrange(
                        "(i p) o -> p i o", p=128))
                    wo_tiles[oc] = wo

                def make(k, oc, t, prefetch_oc):
                    def emit():
                        if oc not in wo_tiles:
                            load_wo(oc)
                        if oc + 1 < OCH and oc + 1 not in wo_tiles:
                            load_wo(oc + 1)
                        if prefetch_oc is not None and prefetch_oc not in wo_tiles:
                            load_wo(prefetch_oc)
                        op_unit(oc, t, wo_tiles[oc], use_dve=(k % 2 == 1))
                    return emit

                k = 0
                for oc in range(OCH):
                    for ti, t in enumerate(range(4 * g, 4 * g + 4)):
                        # prefetch two oc ahead as soon as oc starts
                        pf = oc + 2 if (ti == 0 and oc + 2 < OCH) else None
                        units.append(make(k, oc, t, pf))
                        k += 1

                def prefetch():
                    if 0 not in wo_tiles:
                        load_wo(0)
                    if 1 not in wo_tiles and OCH > 1:
                        load_wo(1)
                return prefetch, units

            # ---- flattened, software-pipelined attention chains for one
            # jc group, with out-proj units interleaved ----
            def emit_group(jc, op_group):
                prefetch_wo, op_units = op_group
                n_kt = 4 * jc + 4
                qsl = slice(jc * 512, (jc + 1) * 512)
                steps = [(h, kt) for h in range(HPC) for kt in range(n_kt)]
                nsteps = len(steps)
                nops = len(op_units)

                chain_ps = {}      # h -> (attn_ps, sums_ps)
                pending = []       # (h, kt, pb, width_off)
                deferred = []      # [countdown, callable]

                def tick():
                    for item in deferred:
                        item[0] -= 1
                    while deferred and deferred[0][0] <= 0:
                        deferred.pop(0)[1]()

                def flush_deferred():
                    while deferred:
                        deferred.pop(0)[1]()

                def emit_front(h, kt):
                    r = kt - 4 * jc
                    o = 0 if r < 0 else 128 * r
                    sc = psum.tile([128, 512], F32, tag="bank", bufs=8)
                    nc.tensor.matmul(sc[:, o:],
                                     qkT[:, HPC, kt * 128:(kt + 1) * 128],
                                     qkT[:, h, jc * 512 + o:(jc + 1) * 512],
                                     start=True, stop=True)
                    pb = pool.tile([128, 512], BF16, tag="pb", bufs=12)
                    nc.scalar.activation(pb[:, o:], sc[:, o:], EXP)
                    if r >= 0:
                        nc.vector.tensor_tensor(pb[:, o:], pb[:, o:],
                                                masks[:, r, o:], mult)
                    pending.append((h, kt, pb, o))

                def emit_back_batch():
                    batch = pending[:4]
                    del pending[:4]
                    h = batch[0][0]
                    if batch[0][1] == 0:
                        attn_ps = psum.tile([128, 512], F32, tag="bank", bufs=8,
                                            name="attn_ps")
                        sums_ps = psum.tile([128, 512], F32, tag="bank", bufs=8,
                                            name="sums_ps")
                        chain_ps[h] = (attn_ps, sums_ps)
                    attn_ps, sums_ps = chain_ps[h]
                    if jc >= 1:
                        # 4 row-sum matmuls packed into distinct 32-wide
                        # column groups of the PE array -- they run
                        # concurrently, each accumulating its kt mod 4
                        # subset into partition 32*g
                        for _, kt, pb, o in batch:
                            g = kt % 4
                            nc.tensor.matmul(sums_ps[32 * g:32 * g + 1, o:],
                                             ones[:, 0:1], pb[:, o:],
                                             start=(kt < 4),
                                             stop=(kt >= n_kt - 4),
                                             skip_group_check=True,
                                             tile_position=(0, 32 * g))
                    else:
                        for _, kt, pb, o in batch:
                            nc.tensor.matmul(sums_ps[0:1, o:],
                                             ones[:, 0:1], pb[:, o:],
                                             start=(kt == 0),
                                             stop=(kt == n_kt - 1),
                                             skip_group_check=True)
                    for _, kt, pb, o in batch:
                        nc.tensor.matmul(attn_ps[:, o:], v_sb[:, kt, :], pb[:, o:],
                                         start=(kt == 0), stop=(kt == n_kt - 1),
                                         skip_group_check=True)
                    if batch[-1][1] == n_kt - 1:
                        if jc >= 1:
                            # chain end: fold the 4 group rows (DVE may read
                            # only one PSUM operand per op), then 1/x
                            t0 = pool.tile([1, 512], F32, tag="fold", bufs=6)
                            nc.vector.tensor_scalar_add(t0[:], sums_ps[0:1, :],
                                                        0.0)
                            t1 = pool.tile([1, 512], F32, tag="fold", bufs=6)
                            nc.vector.tensor_tensor(t1[:], sums_ps[32:33, :],
                                                    t0[:], add)
                            t2 = pool.tile([1, 512], F32, tag="fold", bufs=6)
                            nc.vector.tensor_tensor(t2[:], sums_ps[64:65, :],
                                                    t1[:], add)
                            dsum = pool.tile([1, 512], F32, tag="fold", bufs=6)
                            nc.vector.tensor_tensor(dsum[:], sums_ps[96:97, :],
                                                    t2[:], add)
                        else:
                            dsum = sums_ps[0:1, :]
                        rec = pool.tile([1, 512], F32, tag="rec", bufs=2)
                        nc.vector.reciprocal_approx_fast(rec[:], dsum[:])
                        recb = pool.tile([128, 512], F32, tag="recb", bufs=3)
                        nc.gpsimd.partition_broadcast(recb[:], rec[:])

                        def norm(h=h, attn_ps=attn_ps, recb=recb):
                            nc.vector.tensor_tensor(attnT[:, h, qsl],
                                                    attn_ps[:], recb[:], mult)
                        deferred.append([3, norm])

                prefetch_wo()
                opi = 0
                for i, (h, kt) in enumerate(steps):
                    emit_front(h, kt)
                    tick()
                    # evenly distribute the op units among the steps
                    want = (i + 1) * nops // nsteps
                    while opi < want:
                        op_units[opi]()
                        opi += 1
                        tick()
                    while len(pending) >= 8:
                        emit_back_batch()
                        tick()
                while pending:
                    emit_back_batch()
                    tick()
                while opi < nops:
                    op_units[opi]()
                    opi += 1
                flush_deferred()

            # ---- emission order ----
            qkv_sweep(0, after_d={38: load_tables_a})
            qkv_sweep(1, after_d={20: load_tables_b})
            qkv_sweep(2)
            ropes3 = qkv_sweep(3, defer_ropes=True)
            emit_group(0, (lambda: None, []))          # jc=0 chains (need only sweep 0)
            ropes3()                   # sweep 3 ropes (needed by jc=3)
            emit_group(1, op_group_units(0))
            emit_group(2, op_group_units(1))
            emit_group(3, op_group_units(2))
            pf3, units3 = op_group_units(3)
            pf3()
            for u in units3:
                u()

    nc.compile()
    return nc


def kernel(hidden_states, position_ids, Wqkv, Wout):
    global _compiled
    hidden_states = np.asarray(hidden_states, dtype=np.float32)
    position_ids = np.asarray(position_ids).astype(np.int64)
    Wqkv = np.asarray(Wqkv, dtype=np.float32)
    Wout = np.asarray(Wout, dtype=np.float32)

    if _compiled is None:
        _compiled = _build()
    nc = _compiled

    # host prep: rope tables (from actual position_ids), masks, shards
    scale = HD ** -0.5
    half = HD // 2
    inv_freq = 1.0 / (THETA ** (np.arange(half, dtype=np.float64) / half))
    freqs = position_ids.astype(np.float64)[None, :] * inv_freq[:, None]  # [64, T]
    cos = np.cos(freqs)
    sin = np.sin(freqs)
    cosf = np.concatenate([cos, cos], 0)
    sinf = np.concatenate([-sin, sin], 0)
    cosq = (cosf * scale).astype(np.float16)
    sinq = (sinf * scale).astype(np.float16)
    cosk = cosf.astype(np.float16)
    sink = sinf.astype(np.float16)

    p = np.arange(128)[:, None]
    f = np.arange(512)[None, :]
    masks = np.stack([(f >= 128 * r + p) for r in range(4)]).astype(ml_dtypes.bfloat16)

    hidT = np.ascontiguousarray(hidden_states.T).astype(np.float16)
    ones = np.ones((HD, 8), ml_dtypes.bfloat16)

    q_size = N_HEADS * HD
    in_maps = []
    for c in range(N_CORES):
        qrows = Wqkv[c * HPC * HD:(c + 1) * HPC * HD]
        krows = Wqkv[q_size + c * HD:q_size + (c + 1) * HD]
        vrows = Wqkv[q_size + N_KV * HD + c * HD:q_size + N_KV * HD + (c + 1) * HD]
        wqkvT = np.ascontiguousarray(
            np.concatenate([qrows, krows, vrows], 0).T).astype(np.float16)
        woutT = np.ascontiguousarray(
            Wout[:, c * HPC * HD:(c + 1) * HPC * HD].T).astype(np.float16)
        in_maps.append({
            "hidT": hidT, "wqkvT": wqkvT, "woutT": woutT,
            "cosq": cosq, "sinq": sinq, "cosk": cosk, "sink": sink,
            "maskm": masks, "ones": ones,
        })

    trace = os.environ.get("DBRX_TRACE", "0") == "1"
    res = run_bass_kernel_spmd(nc, in_maps, core_ids=list(range(N_CORES)),
                               trace=trace)
    kernel.last_result = res

    out = res.results[0]["outp"].astype(np.float32)
    for c in range(1, N_CORES):
        out += res.results[c]["outp"]
    return out
